# revision 64
# baseline (speedup 1.0000x reference)
"""Multi-head attention (B=4, S=2048, D=1024, H=16) on 8 Trainium2 NeuronCores.

Sharding: core c = (batch b = c//2, head-group hg = c%2). Each core computes
heads hg*8..hg*8+7 for batch b over the full sequence, producing a partial
output o_c[s, :] = ctx_c @ Wo[:, hg-dims].T (+ bo on hg==0 cores). The host
sums the two partial outputs per batch. This is an exact decomposition: each
core does 1/8 of the total FLOPs with no cross-core communication.

v2 structure (all matmul inputs bf16, accumulation fp32):
  phase 1: KT/QT = W @ z.T feature-major (lhsT = W.T tiles, rhs = z.T tiles);
           V token-major, with a ones column appended per head (softmax
           denominators for free). All 4 pairs' KT/QT stay resident.
  phase 2: blocks (lj, qp) over head pairs x 512-query windows, 16 k-iters
           each. Per iter: the two heads' score matmuls (contraction 64) are
           emitted adjacently at PE row positions (0,0)/(64,0) so they run
           CONCURRENTLY in the array; both land in one [128, 1024] PSUM tile
           (head0 cols 0:512, head1 512:1024) consumed by a single merged
           exp ACT (scale=1/8 fused, max-subtraction dropped -- scores are
           bounded ~N(0,1/3)). PV lags 2 iters and accumulates [65, 512]
           PSUM per head (row 64 = denominator). PSUM: scores 2x2 banks,
           ctx 2x1, misc pool 2x1 = 8, no pool contention.
           bk dropped (softmax shift invariance); bv added post-normalize.
  phase 3: o[s, j] = ctxT.T @ Wo.T partial contraction (+ bo via DVE add);
           query-window w's output tiles run inside block (3, w+1)'s k-loop,
           only window 3 after the last block.
"""

from contextlib import ExitStack

import ml_dtypes
import numpy as np

import concourse.bass as bass
import concourse.tile as tile
from concourse import bacc, library_config, mybir
from concourse.bass_utils import run_bass_kernel_spmd

BF16 = mybir.dt.bfloat16
F32 = mybir.dt.float32
NPBF16 = ml_dtypes.bfloat16

B, S, D, H, DK = 4, 2048, 1024, 16, 64
N_CORES = 8
HG = H // 2  # heads per core
NPAIR = HG // 2  # head pairs per core
ND = D // 128  # contraction d-tiles
NT = S // 128  # token tiles
NQP = 4  # query windows per pair
QW = S // NQP  # query window (512)
DH = HG * DK  # 512: output dims per core
E = DK + 1  # V' columns per head (64 + ones)
SCALE = 1.0 / np.sqrt(DK)
EXP = mybir.ActivationFunctionType.Exp


def _emit(tc, tin, tout):
    nc = tc.nc
    with ExitStack() as ctx:
        SP = ctx.enter_context(tc.tile_pool(name="static", bufs=1))
        SPS = ctx.enter_context(tc.tile_pool(name="spsum", bufs=2, space="PSUM"))
        CPS = ctx.enter_context(tc.tile_pool(name="cpsum", bufs=2, space="PSUM"))
        MPS = ctx.enter_context(tc.tile_pool(name="mpsum", bufs=2, space="PSUM"))
        WKP = ctx.enter_context(tc.tile_pool(name="wkp", bufs=2))
        WQP = ctx.enter_context(tc.tile_pool(name="wqp", bufs=2))
        PP = ctx.enter_context(tc.tile_pool(name="pp", bufs=6))
        DEN = ctx.enter_context(tc.tile_pool(name="denp", bufs=4))
        BCP = ctx.enter_context(tc.tile_pool(name="bcp", bufs=2))
        OSP = ctx.enter_context(tc.tile_pool(name="osp", bufs=4))

        # ---- constants ----
        bq_all = SP.tile([128, NPAIR], F32, tag="bq_all")
        nc.sync.dma_start(bq_all[:], tin["bqc"][:, :])
        bv_all = SP.tile([128, NPAIR], F32, tag="bv_all")
        nc.sync.dma_start(bv_all[:], tin["bvc"][:, :])
        ones64 = SP.tile([128, DK], BF16, tag="ones64")
        nc.vector.memset(ones64[:], 1.0)
        ones128 = SP.tile([1, 128], BF16, tag="ones128")
        nc.vector.memset(ones128[:], 1.0)
        zexp = SP.tile([128, 1], F32, tag="zexp")
        nc.vector.memset(zexp[:], 0.0)

        # ---- static loads ----
        # z.T d-tiles loaded in column-quarters, quarter-major, spread over
        # two DGE streams so the first K/Q projection chunks can start after
        # ~1/4 of z has landed
        dmae = [nc.sync, nc.gpsimd]
        zts = [SP.tile([128, S], BF16, tag=f"zt{d}", name=f"zt{d}") for d in range(ND)]
        wvs = [
            SP.tile([128, DH], BF16, tag=f"wv{d}", name=f"wv{d}") for d in range(ND)
        ]

        def zsl(d, a, b):
            return zts[d][:, a:b]

        def wsl(d):
            return wvs[d][:]

        def load_z_quarter(quarter):
            csl = slice(quarter * 512, (quarter + 1) * 512)
            for d in range(ND):
                dmae[d % 2].dma_start(
                    zts[d][:, csl], tin["ztc"][d * 128 : (d + 1) * 128, csl]
                )

        def load_wv():
            for d in range(ND):
                dmae[d % 2].dma_start(
                    wvs[d][:], tin["wvTc"][d * 128 : (d + 1) * 128, :]
                )

        # V' tiles: [128 tokens, 8 heads x (64 dims + ones col)]
        vsb = []
        for t in range(NT):
            v_ = SP.tile([128, HG * E], BF16, tag=f"vsb{t}", name=f"vsb{t}")
            nc.vector.memset(
                v_.rearrange("p (h e) -> p h e", e=E)[:, :, DK : DK + 1], 1.0
            )
            vsb.append(v_)

        # resident K/Q feature-major tiles, one pair each
        kts = [
            SP.tile([128, S], BF16, tag=f"kt{l}", name=f"kt{l}") for l in range(NPAIR)
        ]
        qts = [
            SP.tile([128, S], BF16, tag=f"qt{l}", name=f"qt{l}") for l in range(NPAIR)
        ]
        ctxu = []
        for lj in range(NPAIR):
            cu = SP.tile([128, S], BF16, tag=f"ctxu{lj}", name=f"ctxu{lj}")
            ctxu.append(cu)

        def emit_vproj(t):
            ps = MPS.tile([128, DH], F32, tag="mps", name=f"psv{t}")
            for d in range(ND):
                nc.tensor.matmul(
                    ps[:],
                    lhsT=zsl(d, t * 128, (t + 1) * 128),
                    rhs=wsl(d),
                    start=(d == 0),
                    stop=(d == ND - 1),
                )
            nc.vector.tensor_copy(
                vsb[t].rearrange("p (h e) -> p h e", e=E)[:, :, 0:DK],
                ps.rearrange("p (h e) -> p h e", e=DK),
            )

        def emit_proj_dmas(lj):
            # weights pre-rearranged on the host: one contiguous 2D DMA per
            # pair brings all 8 d-tiles of the [1024, 128] column block
            jsl = slice(lj * ND * 128, (lj + 1) * ND * 128)
            wkj = WKP.tile([128, ND * 128], BF16, tag="wk", name=f"wk_{lj}")
            nc.sync.dma_start(wkj[:], tin["wkTc"][:, jsl])
            wqj = WQP.tile([128, ND * 128], BF16, tag="wq", name=f"wq_{lj}")
            nc.gpsimd.dma_start(wqj[:], tin["wqTc"][:, jsl])
            return (lj, wkj, wqj)

        def emit_proj_chunk(pst, i):
            """One full K- or Q-projection psum group (8 matmuls + evac).
            i in 0..3: K token chunk i; i in 4..7: Q token chunk i-4."""
            for q in range(4):
                emit_proj_quarter(pst[0], i, q)

        chunk_pend = {}

        def emit_proj_quarter(l, i, quarter):
            """2 of a chunk's 8 accumulating matmuls; evac on the last."""
            lj, wkj, wqj = proj_states[l]
            tcx = i % 4
            key = (l, i)
            if quarter == 0:
                chunk_pend[key] = MPS.tile(
                    [128, 512], F32, tag="mps", name=f"pc{l}_{i}"
                )
            ps = chunk_pend[key]
            w = wkj if i < 4 else wqj
            for d in (2 * quarter, 2 * quarter + 1):
                nc.tensor.matmul(
                    ps[:],
                    lhsT=w[:, d * 128 : (d + 1) * 128],
                    rhs=zsl(d, tcx * 512, (tcx + 1) * 512),
                    start=(d == 0),
                    stop=(d == ND - 1),
                )
            if quarter == 3:
                sl = slice(tcx * 512, (tcx + 1) * 512)
                if i < 4:
                    nc.vector.tensor_copy(kts[l][:, sl], ps[:])
                else:
                    nc.vector.tensor_scalar_add(
                        qts[l][:, sl], ps[:], bq_all[:, l : l + 1]
                    )
                del chunk_pend[key]

        norm_q = []

        def emit_evac(lj, qp, ctx0, ctx1):
            """Boundary: DVE-only evacuation so the ctx PSUM banks free
            without putting PE work behind a DVE dependency. The reciprocal
            reads the denominator row straight out of PSUM."""
            qsl = slice(qp * QW, qp * QW + QW)
            den = DEN.tile([1, 2 * QW], F32, tag="den", name=f"den_{lj}_{qp}")
            nc.vector.tensor_copy(den[0:1, 0:QW], ctx0[64:65, :])
            nc.vector.tensor_copy(den[0:1, QW : 2 * QW], ctx1[64:65, :])
            nc.vector.tensor_copy(ctxu[lj][0:64, qsl], ctx0[0:64, :])
            nc.vector.tensor_copy(ctxu[lj][64:128, qsl], ctx1[0:64, :])
            norm_q.append((lj, qp, den))

        def emit_normalize(lj, qp, den):
            """Deferred into the next block: denominator reciprocal rows are
            broadcast on GPSIMD (keeps both PE and PSUM out of it). The
            reciprocal rows are bf16: halves the broadcast bytes and puts
            the ctxu multiplies in the DVE's 2x packed mode."""
            qsl = slice(qp * QW, qp * QW + QW)
            r = DEN.tile([1, 2 * QW], F32, tag="den", name=f"r_{lj}_{qp}")
            nc.vector.reciprocal_approx_fast(out=r[:], in_=den[:])
            r16 = DEN.tile([1, 2 * QW], BF16, tag="denr", name=f"r16_{lj}_{qp}")
            nc.vector.tensor_copy(r16[:], r[:])
            # partition_broadcast only honors a base-0 output window, so
            # broadcast the combined row full-height and slice per head
            bcr = BCP.tile([128, 2 * QW], BF16, tag="bcr", name=f"bcr{lj}_{qp}")
            nc.gpsimd.partition_broadcast(bcr[:], r16[:])
            nc.vector.tensor_mul(
                ctxu[lj][0:64, qsl], ctxu[lj][0:64, qsl], bcr[0:64, 0:QW]
            )
            nc.vector.tensor_mul(
                ctxu[lj][64:128, qsl], ctxu[lj][64:128, qsl],
                bcr[64:128, QW : 2 * QW],
            )
            nc.vector.tensor_scalar_add(
                ctxu[lj][:, qsl], ctxu[lj][:, qsl], bv_all[:, lj : lj + 1]
            )

        def flush_norms():
            while norm_q:
                emit_normalize(*norm_q.pop(0))

        wos = []
        bo_sb = None
        p3_pend = {}

        def emit_phase3_part(st, jc, half, pool=None, ls=None):
            """2 of an output s-tile column-half's 4 accumulating matmuls.
            half 0 reads ctxu[0:2] (long ready); half 1 reads ctxu[2:4]
            (pair 3's window may be freshly normalized) and evacs + DMAs."""
            ssl = slice(st * 128, (st + 1) * 128)
            jsl = slice(jc * 512, (jc + 1) * 512)
            if jc == 0 and half == 0:
                p3_pend[st] = OSP.tile([128, D], F32, tag="ost", name=f"ost{st}")
            ost = p3_pend[st]
            kps = (st, jc)
            if half == 0:
                p3_pend[kps] = (pool or MPS).tile(
                    [128, 512], F32, tag="sps" if pool is not None else "mps",
                    name=f"pso{st}_{jc}",
                )
            ps = p3_pend[kps]
            for l in (ls if ls is not None else (2 * half, 2 * half + 1)):
                nc.tensor.matmul(
                    ps[:], lhsT=ctxu[l][:, ssl], rhs=wos[l][:, jsl],
                    start=(l == 0), stop=(l == NPAIR - 1),
                )
            if half == 1 and (ls is None or 3 in ls):
                nc.vector.tensor_add(ost[:, jsl], ps[:], bo_sb[:, jsl])
                dmae[jc].dma_start(tout["o"][ssl, jsl], ost[:, jsl])
                del p3_pend[kps]

        def emit_phase3(st):
            for jc in (0, 1):
                for half in (0, 1):
                    emit_phase3_part(st, jc, half)

        def emit_wo_load():
            nonlocal bo_sb
            for pl in range(NPAIR):
                wo_ = SP.tile([128, D], BF16, tag=f"wo{pl}", name=f"wo{pl}")
                nc.sync.dma_start(
                    wo_[:], tin["woTc"][pl * 128 : (pl + 1) * 128, :]
                )
                wos.append(wo_)
            bo_sb = SP.tile([128, D], F32, tag="bo_sb")
            boap = tin["boc"]
            nc.gpsimd.dma_start(
                bo_sb[:],
                bass.AP(
                    tensor=boap.tensor, offset=boap.offset, ap=[[0, 128], [1, D]]
                ),
            )

        # ---- lead-in ----
        proj_states = [None] * NPAIR
        proj_states[0] = emit_proj_dmas(0)
        load_z_quarter(0)
        load_wv()
        for q_ in range(1, 4):
            load_z_quarter(q_)
        # partition_broadcast (normalize) lives in the gpsimd attn
        # library; the ~7us ucode reload sits AFTER the startup DMAs on the
        # gpsimd queue (first broadcast isn't needed until ~60us in)
        nc.gpsimd.load_library(library_config.attn)
        emit_proj_chunk(proj_states[0], 0)  # K tokens 0..511
        emit_proj_chunk(proj_states[0], 4)  # Q tokens 0..511

        def mk_chunk(l, i):
            return lambda: emit_proj_chunk(proj_states[l], i)

        def mk_q(l, i, q):
            return lambda: emit_proj_quarter(l, i, q)

        def mk_dma(l):
            def f():
                proj_states[l] = emit_proj_dmas(l)
            return f

        def mk_p3(st, jc, half):
            return lambda: emit_phase3_part(st, jc, half)

        blocks_list = [(lj, qp) for lj in range(NPAIR) for qp in range(NQP)]

        # block 0 specials, scheduled by k-iter (PE-bound block: vproj has a
        # hard deadline before any PV; K chunks due at iter 4c)
        sched = {bi: {} for bi in range(len(blocks_list))}

        def at(bi, k, thunk):
            sched[bi].setdefault(k, []).append(thunk)

        # vproj shifted +2 (block 0 runs its PV at lag 4, so vsb[t] is due
        # at iter t+3): the first two iterations are scores-only, priming
        # the exp pipeline before the wv DMAs even finish
        for t in range(NT):
            at(0, min(t + 2, NT - 1), lambda t=t: emit_vproj(t))
        for c in (1, 2, 3):
            at(0, 4 * c - 2, mk_chunk(0, c))
        at(0, 13, mk_chunk(0, 5))  # Q window 1

        # lump streams: 2-matmul work units consumed two per iteration-pair
        # AFTER that pair's scores/PV, so surplus PE work never delays the
        # score->exp pipeline. All lumps land strictly before their readers.
        lumps = {bi: [] for bi in range(len(blocks_list))}

        def chunk_lumps(l, i):
            return [mk_q(l, i, q) for q in range(4)]

        # K3 + Q1 of each pair self-host in its own (l,0) block: their
        # deadlines (iter 12 / next window) leave 8+ iterations of margin,
        # and it unloads the previous pair's blocks (K0-K2/Q0 stay hosted
        # one pair ahead -- their deadlines are too tight to self-host)
        lumps[1] = [emit_wo_load] + chunk_lumps(0, 6) + [mk_dma(1)] \
            + chunk_lumps(1, 0) + chunk_lumps(1, 1)
        lumps[2] = chunk_lumps(0, 7)
        lumps[3] = chunk_lumps(1, 4)
        for l in (1, 2, 3):
            lumps[4 * l] = chunk_lumps(l, 2) + chunk_lumps(l, 3) \
                + chunk_lumps(l, 5)
        for l in (2, 3):
            pb = 4 * (l - 1)
            lumps[pb + 1] = chunk_lumps(l - 1, 6) + [mk_dma(l)] \
                + chunk_lumps(l, 0) + chunk_lumps(l, 1)
            lumps[pb + 2] = chunk_lumps(l - 1, 7)
            lumps[pb + 3] = chunk_lumps(l, 4)
        lumps[12] = lumps[12] + chunk_lumps(3, 6) + chunk_lumps(3, 7)
        # hosting blocks carry 24 finer lumps (3/period): only the 2-matmul
        # ctxu[3] parts gate on the fresh window's normalize chain
        for w in (0, 1, 2):
            lumps[13 + w] = [
                mk_p3(st, jc, half) if ls is None else
                (lambda st=st, jc=jc, ls=ls: emit_phase3_part(st, jc, 1, ls=ls))
                for st in range(4 * w, 4 * w + 4)
                for half, ls in ((0, None), (1, (2,)), (1, (3,)))
                for jc in (0, 1)
            ]

        for bi, (lj, qp) in enumerate(blocks_list):
            q0 = qp * QW
            qsl = slice(q0, q0 + QW)
            spread = sched[bi]
            lump_list = lumps[bi]
            li = 0
            ctx0 = CPS.tile([65, QW], F32, tag="cps", name=f"ctx0_{lj}_{qp}")
            ctx1 = CPS.tile([65, QW], F32, tag="cps", name=f"ctx1_{lj}_{qp}")
            lag = []

            def emit_pv(entries):
                for pp_, kk in entries:
                    for hi, ct in ((0, ctx0), (1, ctx1)):
                        h = 2 * lj + hi
                        nc.tensor.matmul(
                            ct[:],
                            lhsT=vsb[kk][:, h * E : h * E + E],
                            rhs=pp_[:, hi * QW : hi * QW + QW],
                            start=(kk == 0), stop=(kk == NT - 1),
                        )

            for k2 in range(NT // 2):
                # scores for both iterations back to back: the PE row-group
                # geometry stays constant so the in-between LDWEIGHTS hide
                pts = []
                for k in (2 * k2, 2 * k2 + 1):
                    ksl = slice(k * 128, (k + 1) * 128)
                    s = SPS.tile([128, 2 * QW], F32, tag="sps", name=f"s_{bi}_{k}")
                    for hp in (0, 64):
                        nc.tensor.matmul(
                            s[:, (hp // 64) * QW : (hp // 64) * QW + QW],
                            lhsT=kts[lj][hp : hp + 64, ksl],
                            rhs=qts[lj][hp : hp + 64, qsl],
                            start=True, stop=True,
                        )
                    p = PP.tile([128, 2 * QW], BF16, tag="pt", name=f"p_{bi}_{k}")
                    nc.scalar.activation(p[:], s[:], EXP, bias=zexp[:], scale=SCALE)
                    pts.append((p, k))
                    # block-0 specials (vproj / K chunks) go AFTER the
                    # iteration's scores so the exp pipeline primes first
                    for fn in spread.get(k, []):
                        fn()
                # previous pair's PV (4 matmuls, constant full-row geometry);
                # block 0 lags one extra period so vproj can trail the wv DMA
                lag.append(pts)
                if len(lag) > (2 if bi == 0 else 1):
                    emit_pv(lag.pop(0))
                # phase-3 hosting blocks flush a period earlier: their
                # ctxu[3]-reading lumps gate on the normalize chain
                if k2 == (0 if bi >= 13 else 1):
                    flush_norms()
                # surplus work: two 2-matmul lumps per iteration-pair
                # (three in the phase-3 hosting blocks, whose lumps are
                # finer-grained)
                for _ in range(3 if len(lump_list) > 16 else 2):
                    if li < len(lump_list):
                        lump_list[li]()
                        li += 1
            for pts in lag:
                emit_pv(pts)
            lag = []
            while li < len(lump_list):
                lump_list[li]()
                li += 1
            emit_evac(lj, qp, ctx0, ctx1)

        # ---- tail: last query window's output projection. With scores
        # finished, SPS banks are free: open four column-groups at once so
        # the ctxu[0:2] matmuls cover window 3's normalize-chain latency ----
        flush_norms()
        for a, b in ((12, 13), (14, 15)):
            # ctxu[0:3] matmuls (independent of the last normalize chain)
            # first; only the 8 ctxu[3] matmuls gate on it
            for jc in (0, 1):
                emit_phase3_part(a, jc, 0)
            for jc in (0, 1):
                emit_phase3_part(b, jc, 0, pool=SPS)
            for jc in (0, 1):
                emit_phase3_part(a, jc, 1, ls=(2,))
            for jc in (0, 1):
                emit_phase3_part(b, jc, 1, ls=(2,))
            for jc in (0, 1):
                emit_phase3_part(a, jc, 1, ls=(3,))
            for jc in (0, 1):
                emit_phase3_part(b, jc, 1, ls=(3,))


def build_nc():
    nc = bacc.Bacc(
        "TRN2", target_bir_lowering=False, debug=False, num_devices=N_CORES
    )
    tin = {
        "ztc": nc.dram_tensor("ztc", [D, S], BF16, kind="ExternalInput").ap(),
        "wqTc": nc.dram_tensor("wqTc", [128, NPAIR * ND * 128], BF16, kind="ExternalInput").ap(),
        "wkTc": nc.dram_tensor("wkTc", [128, NPAIR * ND * 128], BF16, kind="ExternalInput").ap(),
        "wvTc": nc.dram_tensor("wvTc", [D, DH], BF16, kind="ExternalInput").ap(),
        "woTc": nc.dram_tensor("woTc", [DH, D], BF16, kind="ExternalInput").ap(),
        "bqc": nc.dram_tensor("bqc", [128, NPAIR], F32, kind="ExternalInput").ap(),
        "bvc": nc.dram_tensor("bvc", [128, NPAIR], F32, kind="ExternalInput").ap(),
        "boc": nc.dram_tensor("boc", [1, D], F32, kind="ExternalInput").ap(),
    }
    tout = {"o": nc.dram_tensor("o", [S, D], F32, kind="ExternalOutput").ap()}
    with tile.TileContext(nc) as tc:
        _emit(tc, tin, tout)
    nc.compile()
    return nc


_NC = None


def _get_nc():
    global _NC
    if _NC is None:
        _NC = build_nc()
    return _NC


def make_in_maps(z, Wq, bq, Wk, Wv, bv, Wo, bo):
    """Build the 8 per-core input maps from full fp32 inputs."""
    z = np.asarray(z, np.float32)
    bq = np.asarray(bq, np.float32)
    bv = np.asarray(bv, np.float32)
    bo = np.asarray(bo, np.float32)
    wqT = np.asarray(Wq, np.float32).T
    wkT = np.asarray(Wk, np.float32).T
    wvT = np.asarray(Wv, np.float32).T
    woT = np.asarray(Wo, np.float32).T
    zts = [np.ascontiguousarray(z[b].T).astype(NPBF16) for b in range(B)]

    def proj_relayout(wT_hg):
        # [1024 in, 512 out] -> [128, lj, d, j]: out[p, lj*1024 + d*128 + j]
        # = wT[d*128 + p, lj*128 + j], so the device DMA per pair is a
        # single contiguous [128, 1024] slice
        return np.ascontiguousarray(
            wT_hg.reshape(ND, 128, NPAIR, 128)
            .transpose(1, 2, 0, 3)
            .reshape(128, NPAIR * ND * 128)
        )

    per_hg = []
    for hg in range(2):
        dsl = slice(hg * DH, (hg + 1) * DH)
        per_hg.append(
            {
                "wqTc": proj_relayout(wqT[:, dsl]).astype(NPBF16),
                "wkTc": proj_relayout(wkT[:, dsl]).astype(NPBF16),
                "wvTc": np.ascontiguousarray(wvT[:, dsl]).astype(NPBF16),
                "woTc": np.ascontiguousarray(woT[dsl, :]).astype(NPBF16),
                "bqc": np.ascontiguousarray(bq[dsl].reshape(NPAIR, 128).T),
                "bvc": np.ascontiguousarray(bv[dsl].reshape(NPAIR, 128).T),
                "boc": bo.reshape(1, D) if hg == 0 else np.zeros((1, D), np.float32),
            }
        )
    in_maps = []
    for c in range(N_CORES):
        b, hg = c // 2, c % 2
        in_maps.append({"ztc": zts[b], **per_hg[hg]})
    return in_maps


def run(in_maps, trace=False):
    nc = _get_nc()
    return run_bass_kernel_spmd(
        nc, in_maps, core_ids=list(range(N_CORES)), trace=trace
    )


def kernel(z, Wq, bq, Wk, bk, Wv, bv, Wo, bo):
    in_maps = make_in_maps(z, Wq, bq, Wk, Wv, bv, Wo, bo)
    res = run(in_maps)
    out = np.empty((B, S, D), np.float32)
    for b in range(B):
        out[b] = res.results[2 * b]["o"] + res.results[2 * b + 1]["o"]
    return out


# revision 65
# speedup vs baseline: 1.0089x; 1.0089x over previous
"""Multi-head attention (B=4, S=2048, D=1024, H=16) on 8 Trainium2 NeuronCores.

Sharding: core c = (batch b = c//2, head-group hg = c%2). Each core computes
heads hg*8..hg*8+7 for batch b over the full sequence, producing a partial
output o_c[s, :] = ctx_c @ Wo[:, hg-dims].T (+ bo on hg==0 cores). The host
sums the two partial outputs per batch. This is an exact decomposition: each
core does 1/8 of the total FLOPs with no cross-core communication.

v2 structure (all matmul inputs bf16, accumulation fp32):
  phase 1: KT/QT = W @ z.T feature-major (lhsT = W.T tiles, rhs = z.T tiles);
           V token-major, with a ones column appended per head (softmax
           denominators for free). All 4 pairs' KT/QT stay resident.
  phase 2: blocks (lj, qp) over head pairs x 512-query windows, 16 k-iters
           each. Per iter: the two heads' score matmuls (contraction 64) are
           emitted adjacently at PE row positions (0,0)/(64,0) so they run
           CONCURRENTLY in the array; both land in one [128, 1024] PSUM tile
           (head0 cols 0:512, head1 512:1024) consumed by a single merged
           exp ACT (scale=1/8 fused, max-subtraction dropped -- scores are
           bounded ~N(0,1/3)). PV lags 2 iters and accumulates [65, 512]
           PSUM per head (row 64 = denominator). PSUM: scores 2x2 banks,
           ctx 2x1, misc pool 2x1 = 8, no pool contention.
           bk dropped (softmax shift invariance); bv added post-normalize.
  phase 3: o[s, j] = ctxT.T @ Wo.T partial contraction (+ bo via DVE add);
           query-window w's output tiles run inside block (3, w+1)'s k-loop,
           only window 3 after the last block.
"""

from contextlib import ExitStack

import ml_dtypes
import numpy as np

import concourse.bass as bass
import concourse.tile as tile
from concourse import bacc, library_config, mybir
from concourse.bass_utils import run_bass_kernel_spmd

BF16 = mybir.dt.bfloat16
F32 = mybir.dt.float32
NPBF16 = ml_dtypes.bfloat16

B, S, D, H, DK = 4, 2048, 1024, 16, 64
N_CORES = 8
HG = H // 2  # heads per core
NPAIR = HG // 2  # head pairs per core
ND = D // 128  # contraction d-tiles
NT = S // 128  # token tiles
NQP = 4  # query windows per pair
QW = S // NQP  # query window (512)
DH = HG * DK  # 512: output dims per core
E = DK + 1  # V' columns per head (64 + ones)
SCALE = 1.0 / np.sqrt(DK)
EXP = mybir.ActivationFunctionType.Exp


def _emit(tc, tin, tout):
    nc = tc.nc
    with ExitStack() as ctx:
        SP = ctx.enter_context(tc.tile_pool(name="static", bufs=1))
        SPS = ctx.enter_context(tc.tile_pool(name="spsum", bufs=2, space="PSUM"))
        CPS = ctx.enter_context(tc.tile_pool(name="cpsum", bufs=2, space="PSUM"))
        MPS = ctx.enter_context(tc.tile_pool(name="mpsum", bufs=2, space="PSUM"))
        WKP = ctx.enter_context(tc.tile_pool(name="wkp", bufs=2))
        WQP = ctx.enter_context(tc.tile_pool(name="wqp", bufs=2))
        PP = ctx.enter_context(tc.tile_pool(name="pp", bufs=6))
        DEN = ctx.enter_context(tc.tile_pool(name="denp", bufs=4))
        BCP = ctx.enter_context(tc.tile_pool(name="bcp", bufs=2))
        OSP = ctx.enter_context(tc.tile_pool(name="osp", bufs=4))

        # ---- constants ----
        bq_all = SP.tile([128, NPAIR], F32, tag="bq_all")
        nc.sync.dma_start(bq_all[:], tin["bqc"][:, :])
        bv_all = SP.tile([128, NPAIR], F32, tag="bv_all")
        nc.sync.dma_start(bv_all[:], tin["bvc"][:, :])
        ones64 = SP.tile([128, DK], BF16, tag="ones64")
        nc.vector.memset(ones64[:], 1.0)
        ones128 = SP.tile([1, 128], BF16, tag="ones128")
        nc.vector.memset(ones128[:], 1.0)
        zexp = SP.tile([128, 1], F32, tag="zexp")
        nc.vector.memset(zexp[:], 0.0)

        # ---- static loads ----
        # z.T d-tiles loaded in column-quarters, quarter-major, spread over
        # two DGE streams so the first K/Q projection chunks can start after
        # ~1/4 of z has landed
        dmae = [nc.sync, nc.gpsimd]
        zts = [SP.tile([128, S], BF16, tag=f"zt{d}", name=f"zt{d}") for d in range(ND)]
        wvs = [
            SP.tile([128, DH], BF16, tag=f"wv{d}", name=f"wv{d}") for d in range(ND)
        ]

        def zsl(d, a, b):
            return zts[d][:, a:b]

        def wsl(d):
            return wvs[d][:]

        def load_z_quarter(quarter):
            csl = slice(quarter * 512, (quarter + 1) * 512)
            for d in range(ND):
                dmae[d % 2].dma_start(
                    zts[d][:, csl], tin["ztc"][d * 128 : (d + 1) * 128, csl]
                )

        def load_wv():
            for d in range(ND):
                dmae[d % 2].dma_start(
                    wvs[d][:], tin["wvTc"][d * 128 : (d + 1) * 128, :]
                )

        # V' tiles: [128 tokens, 8 heads x (64 dims + ones col)]
        vsb = []
        for t in range(NT):
            v_ = SP.tile([128, HG * E], BF16, tag=f"vsb{t}", name=f"vsb{t}")
            nc.vector.memset(
                v_.rearrange("p (h e) -> p h e", e=E)[:, :, DK : DK + 1], 1.0
            )
            vsb.append(v_)

        # resident K/Q feature-major tiles, one pair each
        kts = [
            SP.tile([128, S], BF16, tag=f"kt{l}", name=f"kt{l}") for l in range(NPAIR)
        ]
        qts = [
            SP.tile([128, S], BF16, tag=f"qt{l}", name=f"qt{l}") for l in range(NPAIR)
        ]
        ctxu = []
        for lj in range(NPAIR):
            cu = SP.tile([128, S], BF16, tag=f"ctxu{lj}", name=f"ctxu{lj}")
            ctxu.append(cu)

        def emit_vproj(t):
            ps = MPS.tile([128, DH], F32, tag="mps", name=f"psv{t}")
            for d in range(ND):
                nc.tensor.matmul(
                    ps[:],
                    lhsT=zsl(d, t * 128, (t + 1) * 128),
                    rhs=wsl(d),
                    start=(d == 0),
                    stop=(d == ND - 1),
                )
            nc.vector.tensor_copy(
                vsb[t].rearrange("p (h e) -> p h e", e=E)[:, :, 0:DK],
                ps.rearrange("p (h e) -> p h e", e=DK),
            )

        def emit_proj_dmas(lj):
            # weights pre-rearranged on the host: one contiguous 2D DMA per
            # pair brings all 8 d-tiles of the [1024, 128] column block
            jsl = slice(lj * ND * 128, (lj + 1) * ND * 128)
            wkj = WKP.tile([128, ND * 128], BF16, tag="wk", name=f"wk_{lj}")
            nc.sync.dma_start(wkj[:], tin["wkTc"][:, jsl])
            wqj = WQP.tile([128, ND * 128], BF16, tag="wq", name=f"wq_{lj}")
            nc.gpsimd.dma_start(wqj[:], tin["wqTc"][:, jsl])
            return (lj, wkj, wqj)

        def emit_proj_chunk(pst, i):
            """One full K- or Q-projection psum group (8 matmuls + evac).
            i in 0..3: K token chunk i; i in 4..7: Q token chunk i-4."""
            for q in range(4):
                emit_proj_quarter(pst[0], i, q)

        chunk_pend = {}

        def emit_proj_quarter(l, i, quarter):
            """2 of a chunk's 8 accumulating matmuls; evac on the last."""
            lj, wkj, wqj = proj_states[l]
            tcx = i % 4
            key = (l, i)
            if quarter == 0:
                chunk_pend[key] = MPS.tile(
                    [128, 512], F32, tag="mps", name=f"pc{l}_{i}"
                )
            ps = chunk_pend[key]
            w = wkj if i < 4 else wqj
            for d in (2 * quarter, 2 * quarter + 1):
                nc.tensor.matmul(
                    ps[:],
                    lhsT=w[:, d * 128 : (d + 1) * 128],
                    rhs=zsl(d, tcx * 512, (tcx + 1) * 512),
                    start=(d == 0),
                    stop=(d == ND - 1),
                )
            if quarter == 3:
                sl = slice(tcx * 512, (tcx + 1) * 512)
                if i < 4:
                    nc.vector.tensor_copy(kts[l][:, sl], ps[:])
                else:
                    nc.vector.tensor_scalar_add(
                        qts[l][:, sl], ps[:], bq_all[:, l : l + 1]
                    )
                del chunk_pend[key]

        norm_q = []

        def emit_evac(lj, qp, ctx0, ctx1):
            """Boundary: DVE-only evacuation so the ctx PSUM banks free
            without putting PE work behind a DVE dependency. The reciprocal
            reads the denominator row straight out of PSUM."""
            qsl = slice(qp * QW, qp * QW + QW)
            den = DEN.tile([1, 2 * QW], F32, tag="den", name=f"den_{lj}_{qp}")
            nc.vector.tensor_copy(den[0:1, 0:QW], ctx0[64:65, :])
            nc.vector.tensor_copy(den[0:1, QW : 2 * QW], ctx1[64:65, :])
            nc.vector.tensor_copy(ctxu[lj][0:64, qsl], ctx0[0:64, :])
            nc.vector.tensor_copy(ctxu[lj][64:128, qsl], ctx1[0:64, :])
            norm_q.append((lj, qp, den))

        def emit_normalize(lj, qp, den):
            """Deferred into the next block: denominator reciprocal rows are
            broadcast on GPSIMD (keeps both PE and PSUM out of it). The
            reciprocal rows are bf16: halves the broadcast bytes and puts
            the ctxu multiplies in the DVE's 2x packed mode."""
            qsl = slice(qp * QW, qp * QW + QW)
            r = DEN.tile([1, 2 * QW], F32, tag="den", name=f"r_{lj}_{qp}")
            nc.vector.reciprocal_approx_fast(out=r[:], in_=den[:])
            r16 = DEN.tile([1, 2 * QW], BF16, tag="denr", name=f"r16_{lj}_{qp}")
            nc.vector.tensor_copy(r16[:], r[:])
            # partition_broadcast only honors a base-0 output window, so
            # broadcast the combined row full-height and slice per head
            bcr = BCP.tile([128, 2 * QW], BF16, tag="bcr", name=f"bcr{lj}_{qp}")
            nc.gpsimd.partition_broadcast(bcr[:], r16[:])
            nc.vector.tensor_mul(
                ctxu[lj][0:64, qsl], ctxu[lj][0:64, qsl], bcr[0:64, 0:QW]
            )
            nc.vector.tensor_mul(
                ctxu[lj][64:128, qsl], ctxu[lj][64:128, qsl],
                bcr[64:128, QW : 2 * QW],
            )
            nc.vector.tensor_scalar_add(
                ctxu[lj][:, qsl], ctxu[lj][:, qsl], bv_all[:, lj : lj + 1]
            )

        def flush_norms():
            while norm_q:
                emit_normalize(*norm_q.pop(0))

        wos = []
        bo_sb = None
        p3_pend = {}

        def emit_phase3_part(st, jc, half, pool=None, ls=None):
            """2 of an output s-tile column-half's 4 accumulating matmuls.
            half 0 reads ctxu[0:2] (long ready); half 1 reads ctxu[2:4]
            (pair 3's window may be freshly normalized) and evacs + DMAs."""
            ssl = slice(st * 128, (st + 1) * 128)
            jsl = slice(jc * 512, (jc + 1) * 512)
            if jc == 0 and half == 0:
                p3_pend[st] = OSP.tile([128, D], F32, tag="ost", name=f"ost{st}")
            ost = p3_pend[st]
            kps = (st, jc)
            if half == 0:
                p3_pend[kps] = (pool or MPS).tile(
                    [128, 512], F32, tag="sps" if pool is not None else "mps",
                    name=f"pso{st}_{jc}",
                )
            ps = p3_pend[kps]
            for l in (ls if ls is not None else (2 * half, 2 * half + 1)):
                nc.tensor.matmul(
                    ps[:], lhsT=ctxu[l][:, ssl], rhs=wos[l][:, jsl],
                    start=(l == 0), stop=(l == NPAIR - 1),
                )
            if half == 1 and (ls is None or 3 in ls):
                nc.vector.tensor_add(ost[:, jsl], ps[:], bo_sb[:, jsl])
                dmae[jc].dma_start(tout["o"][ssl, jsl], ost[:, jsl])
                del p3_pend[kps]

        def emit_phase3(st):
            for jc in (0, 1):
                for half in (0, 1):
                    emit_phase3_part(st, jc, half)

        def emit_wo_load():
            nonlocal bo_sb
            for pl in range(NPAIR):
                wo_ = SP.tile([128, D], BF16, tag=f"wo{pl}", name=f"wo{pl}")
                nc.sync.dma_start(
                    wo_[:], tin["woTc"][pl * 128 : (pl + 1) * 128, :]
                )
                wos.append(wo_)
            bo_sb = SP.tile([128, D], F32, tag="bo_sb")
            boap = tin["boc"]
            nc.gpsimd.dma_start(
                bo_sb[:],
                bass.AP(
                    tensor=boap.tensor, offset=boap.offset, ap=[[0, 128], [1, D]]
                ),
            )

        # ---- lead-in ----
        proj_states = [None] * NPAIR
        proj_states[0] = emit_proj_dmas(0)
        load_z_quarter(0)
        load_wv()
        for q_ in range(1, 4):
            load_z_quarter(q_)
        # partition_broadcast (normalize) lives in the gpsimd attn
        # library; the ~7us ucode reload sits AFTER the startup DMAs on the
        # gpsimd queue (first broadcast isn't needed until ~60us in)
        nc.gpsimd.load_library(library_config.attn)
        emit_proj_chunk(proj_states[0], 0)  # K tokens 0..511
        emit_proj_chunk(proj_states[0], 4)  # Q tokens 0..511

        def mk_chunk(l, i):
            return lambda: emit_proj_chunk(proj_states[l], i)

        def mk_q(l, i, q):
            return lambda: emit_proj_quarter(l, i, q)

        def mk_dma(l):
            def f():
                proj_states[l] = emit_proj_dmas(l)
            return f

        def mk_p3(st, jc, half):
            return lambda: emit_phase3_part(st, jc, half)

        blocks_list = [(lj, qp) for lj in range(NPAIR) for qp in range(NQP)]

        # block 0 specials, scheduled by k-iter (PE-bound block: vproj has a
        # hard deadline before any PV; K chunks due at iter 4c)
        sched = {bi: {} for bi in range(len(blocks_list))}

        def at(bi, k, thunk):
            sched[bi].setdefault(k, []).append(thunk)

        # vproj shifted +2 (block 0 runs its PV at lag 4, so vsb[t] is due
        # at iter t+3): the first two iterations are scores-only, priming
        # the exp pipeline before the wv DMAs even finish
        for t in range(NT):
            at(0, min(t + 2, NT - 1), lambda t=t: emit_vproj(t))
        for c in (1, 2, 3):
            at(0, 4 * c - 2, mk_chunk(0, c))
        at(0, 13, mk_chunk(0, 5))  # Q window 1

        # lump streams: 2-matmul work units consumed two per iteration-pair
        # AFTER that pair's scores/PV, so surplus PE work never delays the
        # score->exp pipeline. All lumps land strictly before their readers.
        lumps = {bi: [] for bi in range(len(blocks_list))}

        def chunk_lumps(l, i):
            return [mk_q(l, i, q) for q in range(4)]

        # K3 + Q1 of each pair self-host in its own (l,0) block: their
        # deadlines (iter 12 / next window) leave 8+ iterations of margin,
        # and it unloads the previous pair's blocks (K0-K2/Q0 stay hosted
        # one pair ahead -- their deadlines are too tight to self-host)
        lumps[1] = [emit_wo_load] + chunk_lumps(0, 6) + [mk_dma(1)] \
            + chunk_lumps(1, 0) + chunk_lumps(1, 1)
        lumps[2] = chunk_lumps(0, 7) + chunk_lumps(1, 2)
        lumps[3] = chunk_lumps(1, 4)
        for l in (1, 2, 3):
            lumps[4 * l] = chunk_lumps(l, 3) + chunk_lumps(l, 5)
        for l in (2, 3):
            pb = 4 * (l - 1)
            lumps[pb + 1] = chunk_lumps(l - 1, 6) + [mk_dma(l)] \
                + chunk_lumps(l, 0) + chunk_lumps(l, 1)
            lumps[pb + 2] = chunk_lumps(l - 1, 7) + chunk_lumps(l, 2)
            lumps[pb + 3] = chunk_lumps(l, 4)
        lumps[12] = lumps[12] + chunk_lumps(3, 6) + chunk_lumps(3, 7)
        for w in (0, 1, 2):
            lumps[13 + w] = [
                mk_p3(st, jc, half)
                for st in range(4 * w, 4 * w + 4)
                for half in (0, 1)
                for jc in (0, 1)
            ]

        for bi, (lj, qp) in enumerate(blocks_list):
            q0 = qp * QW
            qsl = slice(q0, q0 + QW)
            spread = sched[bi]
            lump_list = lumps[bi]
            li = 0
            ctx0 = CPS.tile([65, QW], F32, tag="cps", name=f"ctx0_{lj}_{qp}")
            ctx1 = CPS.tile([65, QW], F32, tag="cps", name=f"ctx1_{lj}_{qp}")
            lag = []

            def emit_pv(entries):
                for pp_, kk in entries:
                    for hi, ct in ((0, ctx0), (1, ctx1)):
                        h = 2 * lj + hi
                        nc.tensor.matmul(
                            ct[:],
                            lhsT=vsb[kk][:, h * E : h * E + E],
                            rhs=pp_[:, hi * QW : hi * QW + QW],
                            start=(kk == 0), stop=(kk == NT - 1),
                        )

            for k2 in range(NT // 2):
                # scores for both iterations back to back: the PE row-group
                # geometry stays constant so the in-between LDWEIGHTS hide
                pts = []
                for k in (2 * k2, 2 * k2 + 1):
                    ksl = slice(k * 128, (k + 1) * 128)
                    s = SPS.tile([128, 2 * QW], F32, tag="sps", name=f"s_{bi}_{k}")
                    for hp in (0, 64):
                        nc.tensor.matmul(
                            s[:, (hp // 64) * QW : (hp // 64) * QW + QW],
                            lhsT=kts[lj][hp : hp + 64, ksl],
                            rhs=qts[lj][hp : hp + 64, qsl],
                            start=True, stop=True,
                        )
                    p = PP.tile([128, 2 * QW], BF16, tag="pt", name=f"p_{bi}_{k}")
                    nc.scalar.activation(p[:], s[:], EXP, bias=zexp[:], scale=SCALE)
                    pts.append((p, k))
                    # block-0 specials (vproj / K chunks) go AFTER the
                    # iteration's scores so the exp pipeline primes first
                    for fn in spread.get(k, []):
                        fn()
                # previous pair's PV (4 matmuls, constant full-row geometry);
                # block 0 lags one extra period so vproj can trail the wv DMA
                lag.append(pts)
                if len(lag) > (2 if bi == 0 else 1):
                    emit_pv(lag.pop(0))
                # phase-3 hosting blocks flush a period earlier: their
                # ctxu[3]-reading lumps gate on the normalize chain
                if k2 == (0 if bi >= 13 else 1):
                    flush_norms()
                # surplus work: two 2-matmul lumps per iteration-pair
                for _ in range(2):
                    if li < len(lump_list):
                        lump_list[li]()
                        li += 1
            for pts in lag:
                emit_pv(pts)
            lag = []
            while li < len(lump_list):
                lump_list[li]()
                li += 1
            emit_evac(lj, qp, ctx0, ctx1)

        # ---- tail: last query window's output projection. With scores
        # finished, SPS banks are free: open four column-groups at once so
        # the ctxu[0:2] matmuls cover window 3's normalize-chain latency ----
        flush_norms()
        for a, b in ((12, 13), (14, 15)):
            # ctxu[0:3] matmuls (independent of the last normalize chain)
            # first; only the 8 ctxu[3] matmuls gate on it
            for jc in (0, 1):
                emit_phase3_part(a, jc, 0)
            for jc in (0, 1):
                emit_phase3_part(b, jc, 0, pool=SPS)
            for jc in (0, 1):
                emit_phase3_part(a, jc, 1, ls=(2,))
            for jc in (0, 1):
                emit_phase3_part(b, jc, 1, ls=(2,))
            for jc in (0, 1):
                emit_phase3_part(a, jc, 1, ls=(3,))
            for jc in (0, 1):
                emit_phase3_part(b, jc, 1, ls=(3,))


def build_nc():
    nc = bacc.Bacc(
        "TRN2", target_bir_lowering=False, debug=False, num_devices=N_CORES
    )
    tin = {
        "ztc": nc.dram_tensor("ztc", [D, S], BF16, kind="ExternalInput").ap(),
        "wqTc": nc.dram_tensor("wqTc", [128, NPAIR * ND * 128], BF16, kind="ExternalInput").ap(),
        "wkTc": nc.dram_tensor("wkTc", [128, NPAIR * ND * 128], BF16, kind="ExternalInput").ap(),
        "wvTc": nc.dram_tensor("wvTc", [D, DH], BF16, kind="ExternalInput").ap(),
        "woTc": nc.dram_tensor("woTc", [DH, D], BF16, kind="ExternalInput").ap(),
        "bqc": nc.dram_tensor("bqc", [128, NPAIR], F32, kind="ExternalInput").ap(),
        "bvc": nc.dram_tensor("bvc", [128, NPAIR], F32, kind="ExternalInput").ap(),
        "boc": nc.dram_tensor("boc", [1, D], F32, kind="ExternalInput").ap(),
    }
    tout = {"o": nc.dram_tensor("o", [S, D], F32, kind="ExternalOutput").ap()}
    with tile.TileContext(nc) as tc:
        _emit(tc, tin, tout)
    nc.compile()
    return nc


_NC = None


def _get_nc():
    global _NC
    if _NC is None:
        _NC = build_nc()
    return _NC


def make_in_maps(z, Wq, bq, Wk, Wv, bv, Wo, bo):
    """Build the 8 per-core input maps from full fp32 inputs."""
    z = np.asarray(z, np.float32)
    bq = np.asarray(bq, np.float32)
    bv = np.asarray(bv, np.float32)
    bo = np.asarray(bo, np.float32)
    wqT = np.asarray(Wq, np.float32).T
    wkT = np.asarray(Wk, np.float32).T
    wvT = np.asarray(Wv, np.float32).T
    woT = np.asarray(Wo, np.float32).T
    zts = [np.ascontiguousarray(z[b].T).astype(NPBF16) for b in range(B)]

    def proj_relayout(wT_hg):
        # [1024 in, 512 out] -> [128, lj, d, j]: out[p, lj*1024 + d*128 + j]
        # = wT[d*128 + p, lj*128 + j], so the device DMA per pair is a
        # single contiguous [128, 1024] slice
        return np.ascontiguousarray(
            wT_hg.reshape(ND, 128, NPAIR, 128)
            .transpose(1, 2, 0, 3)
            .reshape(128, NPAIR * ND * 128)
        )

    per_hg = []
    for hg in range(2):
        dsl = slice(hg * DH, (hg + 1) * DH)
        per_hg.append(
            {
                "wqTc": proj_relayout(wqT[:, dsl]).astype(NPBF16),
                "wkTc": proj_relayout(wkT[:, dsl]).astype(NPBF16),
                "wvTc": np.ascontiguousarray(wvT[:, dsl]).astype(NPBF16),
                "woTc": np.ascontiguousarray(woT[dsl, :]).astype(NPBF16),
                "bqc": np.ascontiguousarray(bq[dsl].reshape(NPAIR, 128).T),
                "bvc": np.ascontiguousarray(bv[dsl].reshape(NPAIR, 128).T),
                "boc": bo.reshape(1, D) if hg == 0 else np.zeros((1, D), np.float32),
            }
        )
    in_maps = []
    for c in range(N_CORES):
        b, hg = c // 2, c % 2
        in_maps.append({"ztc": zts[b], **per_hg[hg]})
    return in_maps


def run(in_maps, trace=False):
    nc = _get_nc()
    return run_bass_kernel_spmd(
        nc, in_maps, core_ids=list(range(N_CORES)), trace=trace
    )


def kernel(z, Wq, bq, Wk, bk, Wv, bv, Wo, bo):
    in_maps = make_in_maps(z, Wq, bq, Wk, Wv, bv, Wo, bo)
    res = run(in_maps)
    out = np.empty((B, S, D), np.float32)
    for b in range(B):
        out[b] = res.results[2 * b]["o"] + res.results[2 * b + 1]["o"]
    return out
